# revision 2
# baseline (speedup 1.0000x reference)
"""Trainium2 Bass kernel for nn_Conv2dModulated (modulated transposed conv + blur).

Math restructure (validated vs reference):
  s = w @ affine_w.T + affine_b + 1                    (B, CIN)  host
  d = rsqrt(s^2 @ sum_kk(W^2).T + 1e-8)               (B, COUT) host
  out[b] = d[b,:]/16 * blur(convT2x(s[b,:] * x[b], W)) + bias
- Modulation folds into x (per-input-channel scale), demodulation into the
  PSUM eviction (per-output-channel scale) -> weights stay sample-independent.
- Stride-2 transposed conv = 4 parity classes of <=2x2-tap convs on the 32x32
  input (subpixel decomposition; 9 effective taps instead of 36 dilated).
- The 4x4 blur ([1,3,3,1] x [1,3,3,1])/16 = three [1,1] adds per dim on DVE
  in bf16 (2x mode); the final add runs on GPSIMD (Pool) to unload DVE; /16
  folded into d; bias added on the host (linear post-op).
- Output DMA'd in bf16, converted to fp32 on the host.

Sharding: data-parallel over batch, 2 samples per core, 8 cores, no
collectives.
"""

import os
from contextlib import ExitStack

import numpy as np
import ml_dtypes

import concourse.bass as bass
import concourse.tile as tile
from concourse import mybir
from concourse.bass_utils import run_bass_kernel_spmd

B, CIN, COUT, LAT, H, W_SP, KK = 16, 512, 512, 512, 32, 32, 3
NCORES = 8
BPC = B // NCORES  # samples per core
P = 128
NCI = CIN // P
NCO = COUT // P
BF16 = mybir.dt.bfloat16
F32 = mybir.dt.float32
ADD = mybir.AluOpType.add

# tap order by first use: class (0,0) -> (0,1) -> (1,0) -> (1,1)
TAP_ORDER = [0, 2, 6, 8, 1, 7, 3, 5, 4]

_ENG_PREFIX = {
    "PE": "PE_", "DVE": "DVE_", "Activation": "Activation_",
    "Pool": "Pool_", "SP": "SP_",
}


def _fix_waits(nc: bass.Bass) -> None:
    """Walrus codegen accepts only one sem-wait per compute instruction;
    Tile emits up to 4.

    1) Drop same-engine self-waits: every engine executes its stream
       serially in order, so a wait on the engine's own completion
       semaphore is redundant.
    2) Split any remaining multi-wait onto same-engine NoOp instructions
       inserted just before the instruction.
    """
    for f in nc.m.functions:
        for bb in f.blocks:
            out = []
            for inst in bb.instructions:
                si = inst.sync_info
                if si is None or len(si.on_wait) <= 1:
                    out.append(inst)
                    continue
                eng = str(inst.engine).split(".")[-1]
                pfx = _ENG_PREFIX.get(eng)
                waits = list(si.on_wait)
                keep = [
                    w for w in waits
                    if not (pfx and (w.ant_name or "").startswith(pfx))
                ]
                for w in keep[:-1]:
                    nop = mybir.InstNoOp(name=nc.get_next_instruction_name())
                    nop.engine = inst.engine
                    nop.sync_info = mybir.SyncInfo(on_wait=[w], on_update=[])
                    out.append(nop)
                keep = keep[-1:]
                inst.sync_info = mybir.SyncInfo(
                    on_wait=keep, on_update=list(si.on_update)
                )
                out.append(inst)
            bb.instructions = out


def build_program() -> bass.Bass:
    nc = bass.Bass()
    xp_d = nc.declare_dram_parameter("xp", [BPC, NCI, P, 34 * 34], BF16, isOutput=False)
    wt_d = nc.declare_dram_parameter("wt", [NCI, P, 9 * COUT], BF16, isOutput=False)
    dsc_d = nc.declare_dram_parameter("dsc", [P, BPC * NCO], F32, isOutput=False)
    out_d = nc.declare_dram_parameter("out", [BPC, NCO, P, 64 * 64], BF16, isOutput=True)

    with ExitStack() as ctx:
        tc = ctx.enter_context(tile.TileContext(nc))
        consts = ctx.enter_context(tc.tile_pool(name="consts", bufs=1))
        xpool = ctx.enter_context(tc.tile_pool(name="xpool", bufs=1))
        psum = ctx.enter_context(tc.tile_pool(name="psum", bufs=8, space="PSUM"))
        spool = ctx.enter_context(tc.tile_pool(name="spool", bufs=6))
        opool = ctx.enter_context(tc.tile_pool(name="opool", bufs=2))

        d_sb = consts.tile([P, BPC * NCO], F32, tag="dsb")
        nc.sync.dma_start(out=d_sb[:], in_=dsc_d[:])

        # Warmup: ramp PE p-state on a zeroed tile while input DMAs stream.
        warm_x = consts.tile([P, 512], BF16, tag="warmx")
        nc.vector.memset(warm_x[:], 0.0)
        for wi in range(6):
            pwarm = psum.tile([P, 512], F32, tag="ps", name=f"pswarm{wi}")
            nc.tensor.matmul(
                pwarm[:], warm_x[:, 0:P], warm_x[:], start=True, stop=True
            )

        # Input DMAs in first-use order: per cin-block interleave x(s0) with
        # the taps needed by the first parity class, then the rest.
        w_sb = consts.tile([P, NCI, 9 * COUT], BF16, tag="wsb")
        x_tiles = {}
        for c in range(NCI):
            t = xpool.tile([P, 34, 34], BF16, tag=f"x0{c}")
            nc.sync.dma_start(
                out=t[:], in_=xp_d[0, c].rearrange("p (a b) -> p a b", b=34)
            )
            x_tiles[(0, c)] = t
            for tp in TAP_ORDER[:4]:
                nc.sync.dma_start(
                    out=w_sb[:, c, tp * COUT : (tp + 1) * COUT],
                    in_=wt_d[c][:, tp * COUT : (tp + 1) * COUT],
                )
        for tp in TAP_ORDER[4:]:
            for c in range(NCI):
                nc.sync.dma_start(
                    out=w_sb[:, c, tp * COUT : (tp + 1) * COUT],
                    in_=wt_d[c][:, tp * COUT : (tp + 1) * COUT],
                )
        for c in range(NCI):
            t = xpool.tile([P, 34, 34], BF16, tag=f"x1{c}")
            nc.sync.dma_start(
                out=t[:], in_=xp_d[1, c].rearrange("p (a b) -> p a b", b=34)
            )
            x_tiles[(1, c)] = t

        # Engine warm-up ops that absorb DMA-completion waits.
        warm_a = consts.tile([P, 1], F32, tag="warm_a")
        nc.scalar.copy(warm_a[:], d_sb[:, 0:1])
        warm_v = consts.tile([P, 1], F32, tag="warm_v")
        nc.vector.tensor_copy(warm_v[:], d_sb[:, 0:1])

        # Persistent padded y65 buffers; zeroed once on ACT (borders stay
        # zero; the interior is fully overwritten by every eviction round).
        y_tiles = []
        for i in range(3):
            yt = consts.tile([P, 67 * 67 + 1], BF16, tag=f"ybuf{i}")
            nc.scalar.memzero(yt[:])
            y_tiles.append(yt)

        units = [(s, oc) for s in range(BPC) for oc in range(NCO)]
        n_units = len(units)

        def h_chain(y65p, rows, tag_sfx):
            """Horizontal [1,1]x3 chain over a row slice (slice(None) = all)."""
            s1 = spool.tile([P, 67, 66], BF16, tag="scr", name=f"s1{tag_sfx}")
            nc.vector.tensor_add(
                s1[:, rows, :], y65p[:, rows, 0:66], y65p[:, rows, 1:67]
            )
            s2 = spool.tile([P, 67, 65], BF16, tag="scr", name=f"s2{tag_sfx}")
            nc.vector.tensor_add(
                s2[:, rows, :], s1[:, rows, 0:65], s1[:, rows, 1:66]
            )
            return s1, s2

        for ui, (s, oc) in enumerate(units):
            last = ui == n_units - 1
            y65p = y_tiles[ui % 3][:, 0 : 67 * 67].rearrange(
                "p (a b) -> p a b", b=67
            )
            zz = spool.tile([P, 67, 64], BF16, tag="scr", name=f"zz{ui}")
            for eh, ec in ((0, 0), (0, 1), (1, 0), (1, 1)):
                rtaps = [(0, 0), (2, 1)] if eh == 0 else [(1, 1)]
                ctaps = [(0, 0), (2, 1)] if ec == 0 else [(1, 1)]
                ncols = 33 if ec == 0 else 32
                if eh == 0:
                    rchunks = [(0, 11), (11, 11), (22, 11)]
                elif ec == 0:
                    rchunks = [(0, 11), (11, 11), (22, 10)]
                else:
                    rchunks = [(0, 16), (16, 16)]
                taps = [(kh, kw, ra, cb) for (kh, ra) in rtaps for (kw, cb) in ctaps]
                ptiles = [
                    psum.tile([P, 512], F32, tag="ps", name=f"ps{s}{oc}{eh}{ec}{fc}")
                    for fc in range(len(rchunks))
                ]
                nmm = len(taps) * NCI
                i = 0
                for c in range(NCI):
                    for kh, kw, ra, cb in taps:
                        toff = (kh * 3 + kw) * COUT + oc * P
                        lhsT = w_sb[:, c, toff : toff + P]
                        for fc, (u0, nr) in enumerate(rchunks):
                            rhs = x_tiles[(s, c)][:, u0 + ra : u0 + ra + nr,
                                                  cb : cb + ncols]
                            nc.tensor.matmul(
                                ptiles[fc][:, : nr * ncols], lhsT, rhs,
                                start=(i == 0), stop=(i == nmm - 1),
                            )
                        i += 1
                for fc, (u0, nr) in enumerate(rchunks):
                    src = ptiles[fc][:, : nr * ncols].rearrange(
                        "p (r c) -> p r c", c=ncols
                    )
                    dst = y65p[:, 1 + 2 * u0 + eh : 1 + 2 * (u0 + nr) + eh : 2,
                               1 + ec : 1 + 2 * ncols + ec : 2]
                    nc.scalar.activation(
                        dst, src, mybir.ActivationFunctionType.Copy,
                        bias=0.0,
                        scale=d_sb[:, s * NCO + oc : s * NCO + oc + 1],
                    )
                if last and (eh, ec) == (0, 1):
                    # blur odd y65p rows (parity eh=0) while (1,*) matmuls run
                    s1t, s2t = h_chain(y65p, slice(1, 66, 2), f"t{ui}")
                    nc.vector.tensor_add(
                        zz[:, 1:66:2, :], s2t[:, 1:66:2, 0:64], s2t[:, 1:66:2, 1:65]
                    )

            if last:
                s1b, s2b = h_chain(y65p, slice(0, 67, 2), f"b{ui}")
                nc.vector.tensor_add(
                    zz[:, 0:67:2, :], s2b[:, 0:67:2, 0:64], s2b[:, 0:67:2, 1:65]
                )
            else:
                s1, s2 = h_chain(y65p, slice(None), f"{ui}")
                nc.vector.tensor_add(zz[:], s2[:, :, 0:64], s2[:, :, 1:65])

            c1 = spool.tile([P, 66, 64], BF16, tag="scr", name=f"c1{ui}")
            nc.vector.tensor_add(c1[:], zz[:, 0:66, :], zz[:, 1:67, :])
            c2 = spool.tile([P, 65, 64], BF16, tag="scr", name=f"c2{ui}")
            nc.vector.tensor_add(c2[:], c1[:, 0:65, :], c1[:, 1:66, :])
            of = opool.tile([P, 64, 64], BF16, tag="out")
            if last:
                # DVE + split halves: shortest possible tail
                nc.vector.tensor_add(
                    of[:, 0:32, :], c2[:, 0:32, :], c2[:, 1:33, :]
                )
                nc.sync.dma_start(
                    out=out_d[s, oc][:, 0 : 32 * 64],
                    in_=of[:, 0:32, :].rearrange("p a b -> p (a b)"),
                )
                nc.vector.tensor_add(
                    of[:, 32:64, :], c2[:, 32:64, :], c2[:, 33:65, :]
                )
                nc.sync.dma_start(
                    out=out_d[s, oc][:, 32 * 64 : 64 * 64],
                    in_=of[:, 32:64, :].rearrange("p a b -> p (a b)"),
                )
            else:
                nc.gpsimd.tensor_add(of[:], c2[:, 0:64, :], c2[:, 1:65, :])
                nc.sync.dma_start(
                    out=out_d[s, oc],
                    in_=of[:].rearrange("p a b -> p (a b)"),
                )
    _fix_waits(nc)
    return nc


def make_in_maps(x, w, weight, bias, affine_w, affine_b):
    x = np.asarray(x, np.float32)
    w = np.asarray(w, np.float32)
    weight = np.asarray(weight, np.float32)
    affine_w = np.asarray(affine_w, np.float32)
    affine_b = np.asarray(affine_b, np.float32)

    s = w @ affine_w.T + affine_b + 1.0  # (B, CIN)
    wsq = (weight.astype(np.float64) ** 2).sum(axis=(2, 3))  # (COUT, CIN)
    d = 1.0 / np.sqrt((s.astype(np.float64) ** 2) @ wsq.T + 1e-8)  # (B, COUT)
    d16 = (d / 16.0).astype(np.float32)

    xp = np.zeros((B, CIN, 34, 34), np.float32)
    xp[:, :, 1:33, 1:33] = x * s[:, :, None, None]
    xp_bf = xp.astype(ml_dtypes.bfloat16).reshape(B, NCI, P, 34 * 34)

    wf = weight[:, :, ::-1, ::-1]  # spatial flip
    wt = np.ascontiguousarray(
        wf.transpose(1, 2, 3, 0).reshape(NCI, P, 9 * COUT)
    ).astype(ml_dtypes.bfloat16)

    in_maps = []
    for core in range(NCORES):
        sl = slice(core * BPC, (core + 1) * BPC)
        dcore = d16[sl].reshape(BPC, NCO, P)
        dsc = np.ascontiguousarray(dcore.transpose(2, 0, 1).reshape(P, BPC * NCO))
        in_maps.append(
            {
                "xp": np.ascontiguousarray(xp_bf[sl]),
                "wt": wt,
                "dsc": dsc,
            }
        )
    return in_maps


LAST_RESULTS = None  # BassKernelResults of the most recent run (for test harness)


def kernel(x, w, weight, bias, affine_w, affine_b):
    global LAST_RESULTS
    in_maps = make_in_maps(x, w, weight, bias, affine_w, affine_b)
    nc = build_program()
    res = run_bass_kernel_spmd(nc, in_maps, list(range(NCORES)))
    LAST_RESULTS = res
    outs = [
        np.asarray(r["out"]).astype(np.float32).reshape(BPC, COUT, 64, 64)
        for r in res.results
    ]
    full = np.concatenate(outs, axis=0)
    full += np.asarray(bias, np.float32)  # (1, COUT, 1, 1) broadcast
    return np.ascontiguousarray(full, dtype=np.float32)


# revision 4
# speedup vs baseline: 1.1650x; 1.1650x over previous
"""Trainium2 Bass kernel for nn_Conv2dModulated (modulated transposed conv + blur).

Math restructure (validated vs reference):
  s = w @ affine_w.T + affine_b + 1                    (B, CIN)  host
  d = rsqrt(s^2 @ sum_kk(W^2).T + 1e-8)               (B, COUT) host
  out[b] = d[b,:]/16 * blur(convT2x(s[b,:] * x[b], W)) + bias
- Modulation folds into x (per-input-channel scale), demodulation into the
  PSUM eviction (per-output-channel scale) -> weights stay sample-independent.
- Stride-2 transposed conv = 4 parity classes of <=2x2-tap convs on the 32x32
  input (subpixel decomposition; 9 effective taps instead of 36 dilated).
- The 4x4 blur ([1,3,3,1] x [1,3,3,1])/16 = three [1,1] adds per dim on DVE
  in bf16 (2x mode); the final add runs on GPSIMD (Pool) to unload DVE; /16
  folded into d; bias added on the host (linear post-op).
- Output DMA'd in bf16, converted to fp32 on the host.

Sharding: data-parallel over batch, 2 samples per core, 8 cores, no
collectives.
"""

import os
from contextlib import ExitStack

import numpy as np
import ml_dtypes

import concourse.bass as bass
import concourse.tile as tile
from concourse import mybir
from concourse.bass_utils import run_bass_kernel_spmd

B, CIN, COUT, LAT, H, W_SP, KK = 16, 512, 512, 512, 32, 32, 3
NCORES = 8
BPC = B // NCORES  # samples per core
P = 128
NCI = CIN // P
NCO = COUT // P
BF16 = mybir.dt.bfloat16
F32 = mybir.dt.float32
ADD = mybir.AluOpType.add

# tap order by first use: class (0,0) -> (0,1) -> (1,0) -> (1,1)
TAP_ORDER = [0, 2, 6, 8, 1, 7, 3, 5, 4]

_ENG_PREFIX = {
    "PE": "PE_", "DVE": "DVE_", "Activation": "Activation_",
    "Pool": "Pool_", "SP": "SP_",
}


def _fix_waits(nc: bass.Bass) -> None:
    """Walrus codegen accepts only one sem-wait per compute instruction;
    Tile emits up to 4.

    1) Drop same-engine self-waits: every engine executes its stream
       serially in order, so a wait on the engine's own completion
       semaphore is redundant.
    2) Split any remaining multi-wait onto same-engine NoOp instructions
       inserted just before the instruction.
    """
    for f in nc.m.functions:
        for bb in f.blocks:
            out = []
            for inst in bb.instructions:
                si = inst.sync_info
                if si is None or len(si.on_wait) <= 1:
                    out.append(inst)
                    continue
                eng = str(inst.engine).split(".")[-1]
                pfx = _ENG_PREFIX.get(eng)
                waits = list(si.on_wait)
                keep = [
                    w for w in waits
                    if not (pfx and (w.ant_name or "").startswith(pfx))
                ]
                for w in keep[:-1]:
                    nop = mybir.InstNoOp(name=nc.get_next_instruction_name())
                    nop.engine = inst.engine
                    nop.sync_info = mybir.SyncInfo(on_wait=[w], on_update=[])
                    out.append(nop)
                keep = keep[-1:]
                inst.sync_info = mybir.SyncInfo(
                    on_wait=keep, on_update=list(si.on_update)
                )
                out.append(inst)
            bb.instructions = out


def build_program() -> bass.Bass:
    nc = bass.Bass()
    xp_d = nc.declare_dram_parameter("xp", [BPC, NCI, P, 34 * 34], BF16, isOutput=False)
    wt_d = nc.declare_dram_parameter("wt", [NCI, P, 9 * COUT], BF16, isOutput=False)
    dsc_d = nc.declare_dram_parameter("dsc", [P, BPC * NCO], F32, isOutput=False)
    out_d = nc.declare_dram_parameter("out", [BPC, NCO, P, 64 * 64], BF16, isOutput=True)

    with ExitStack() as ctx:
        tc = ctx.enter_context(tile.TileContext(nc))
        consts = ctx.enter_context(tc.tile_pool(name="consts", bufs=1))
        xpool = ctx.enter_context(tc.tile_pool(name="xpool", bufs=1))
        psum = ctx.enter_context(tc.tile_pool(name="psum", bufs=8, space="PSUM"))
        spool = ctx.enter_context(tc.tile_pool(name="spool", bufs=6))
        opool = ctx.enter_context(tc.tile_pool(name="opool", bufs=2))

        d_sb = consts.tile([P, BPC * NCO], F32, tag="dsb")
        nc.sync.dma_start(out=d_sb[:], in_=dsc_d[:])

        # Warmup: ramp PE p-state on a zeroed tile while input DMAs stream.
        warm_x = consts.tile([P, 512], BF16, tag="warmx")
        nc.vector.memset(warm_x[:], 0.0)
        for wi in range(6):
            pwarm = psum.tile([P, 512], F32, tag="ps", name=f"pswarm{wi}")
            nc.tensor.matmul(
                pwarm[:], warm_x[:, 0:P], warm_x[:], start=True, stop=True
            )

        # Input DMAs in first-use order: x(s0,c) then the full weight block
        # for that cin-block (big contiguous transfers issue cheaply).
        w_sb = consts.tile([P, NCI, 9 * COUT], BF16, tag="wsb")
        x_tiles = {}
        for c in range(NCI):
            t = xpool.tile([P, 34, 34], BF16, tag=f"x0{c}")
            nc.sync.dma_start(
                out=t[:], in_=xp_d[0, c].rearrange("p (a b) -> p a b", b=34)
            )
            x_tiles[(0, c)] = t
            nc.sync.dma_start(out=w_sb[:, c, :], in_=wt_d[c])
        for c in range(NCI):
            t = xpool.tile([P, 34, 34], BF16, tag=f"x1{c}")
            nc.sync.dma_start(
                out=t[:], in_=xp_d[1, c].rearrange("p (a b) -> p a b", b=34)
            )
            x_tiles[(1, c)] = t

        # Engine warm-up ops that absorb DMA-completion waits.
        warm_a = consts.tile([P, 1], F32, tag="warm_a")
        nc.scalar.copy(warm_a[:], d_sb[:, 0:1])
        warm_v = consts.tile([P, 1], F32, tag="warm_v")
        nc.vector.tensor_copy(warm_v[:], d_sb[:, 0:1])

        # Persistent padded y65 buffers; zeroed once on ACT (borders stay
        # zero; the interior is fully overwritten by every eviction round).
        y_tiles = []
        for i in range(3):
            yt = consts.tile([P, 67 * 67 + 1], BF16, tag=f"ybuf{i}")
            nc.scalar.memzero(yt[:])
            y_tiles.append(yt)

        units = [(s, oc) for s in range(BPC) for oc in range(NCO)]
        n_units = len(units)

        def h_chain(y65p, rows, tag_sfx):
            """Horizontal [1,1]x3 chain over a row slice (slice(None) = all)."""
            s1 = spool.tile([P, 67, 66], BF16, tag="scr", name=f"s1{tag_sfx}")
            nc.vector.tensor_add(
                s1[:, rows, :], y65p[:, rows, 0:66], y65p[:, rows, 1:67]
            )
            s2 = spool.tile([P, 67, 65], BF16, tag="scr", name=f"s2{tag_sfx}")
            nc.vector.tensor_add(
                s2[:, rows, :], s1[:, rows, 0:65], s1[:, rows, 1:66]
            )
            return s1, s2

        for ui, (s, oc) in enumerate(units):
            last = ui == n_units - 1
            y65p = y_tiles[ui % 3][:, 0 : 67 * 67].rearrange(
                "p (a b) -> p a b", b=67
            )
            zz = spool.tile([P, 67, 64], BF16, tag="scr", name=f"zz{ui}")
            for eh, ec in ((0, 0), (0, 1), (1, 0), (1, 1)):
                rtaps = [(0, 0), (2, 1)] if eh == 0 else [(1, 1)]
                ctaps = [(0, 0), (2, 1)] if ec == 0 else [(1, 1)]
                ncols = 33 if ec == 0 else 32
                if eh == 0:
                    rchunks = [(0, 11), (11, 11), (22, 11)]
                elif ec == 0:
                    rchunks = [(0, 11), (11, 11), (22, 10)]
                else:
                    rchunks = [(0, 16), (16, 16)]
                taps = [(kh, kw, ra, cb) for (kh, ra) in rtaps for (kw, cb) in ctaps]
                ptiles = [
                    psum.tile([P, 512], F32, tag="ps", name=f"ps{s}{oc}{eh}{ec}{fc}")
                    for fc in range(len(rchunks))
                ]
                nmm = len(taps) * NCI
                i = 0
                for c in range(NCI):
                    for kh, kw, ra, cb in taps:
                        toff = (kh * 3 + kw) * COUT + oc * P
                        lhsT = w_sb[:, c, toff : toff + P]
                        for fc, (u0, nr) in enumerate(rchunks):
                            rhs = x_tiles[(s, c)][:, u0 + ra : u0 + ra + nr,
                                                  cb : cb + ncols]
                            nc.tensor.matmul(
                                ptiles[fc][:, : nr * ncols], lhsT, rhs,
                                start=(i == 0), stop=(i == nmm - 1),
                            )
                        i += 1
                for fc, (u0, nr) in enumerate(rchunks):
                    src = ptiles[fc][:, : nr * ncols].rearrange(
                        "p (r c) -> p r c", c=ncols
                    )
                    dst = y65p[:, 1 + 2 * u0 + eh : 1 + 2 * (u0 + nr) + eh : 2,
                               1 + ec : 1 + 2 * ncols + ec : 2]
                    nc.scalar.activation(
                        dst, src, mybir.ActivationFunctionType.Copy,
                        bias=0.0,
                        scale=d_sb[:, s * NCO + oc : s * NCO + oc + 1],
                    )
                if last and (eh, ec) == (0, 1):
                    # blur odd y65p rows (parity eh=0) while (1,*) matmuls run
                    s1t, s2t = h_chain(y65p, slice(1, 66, 2), f"t{ui}")
                    nc.vector.tensor_add(
                        zz[:, 1:66:2, :], s2t[:, 1:66:2, 0:64], s2t[:, 1:66:2, 1:65]
                    )

            if last:
                s1b, s2b = h_chain(y65p, slice(0, 67, 2), f"b{ui}")
                nc.vector.tensor_add(
                    zz[:, 0:67:2, :], s2b[:, 0:67:2, 0:64], s2b[:, 0:67:2, 1:65]
                )
            else:
                s1, s2 = h_chain(y65p, slice(None), f"{ui}")
                nc.vector.tensor_add(zz[:], s2[:, :, 0:64], s2[:, :, 1:65])

            c1 = spool.tile([P, 66, 64], BF16, tag="scr", name=f"c1{ui}")
            nc.vector.tensor_add(c1[:], zz[:, 0:66, :], zz[:, 1:67, :])
            c2 = spool.tile([P, 65, 64], BF16, tag="scr", name=f"c2{ui}")
            nc.vector.tensor_add(c2[:], c1[:, 0:65, :], c1[:, 1:66, :])
            of = opool.tile([P, 64, 64], BF16, tag="out")
            if last:
                # DVE + split halves: shortest possible tail
                nc.vector.tensor_add(
                    of[:, 0:32, :], c2[:, 0:32, :], c2[:, 1:33, :]
                )
                nc.sync.dma_start(
                    out=out_d[s, oc][:, 0 : 32 * 64],
                    in_=of[:, 0:32, :].rearrange("p a b -> p (a b)"),
                )
                nc.vector.tensor_add(
                    of[:, 32:64, :], c2[:, 32:64, :], c2[:, 33:65, :]
                )
                nc.sync.dma_start(
                    out=out_d[s, oc][:, 32 * 64 : 64 * 64],
                    in_=of[:, 32:64, :].rearrange("p a b -> p (a b)"),
                )
            else:
                nc.vector.tensor_add(of[:], c2[:, 0:64, :], c2[:, 1:65, :])
                nc.sync.dma_start(
                    out=out_d[s, oc],
                    in_=of[:].rearrange("p a b -> p (a b)"),
                )
    _fix_waits(nc)
    return nc


def make_in_maps(x, w, weight, bias, affine_w, affine_b):
    x = np.asarray(x, np.float32)
    w = np.asarray(w, np.float32)
    weight = np.asarray(weight, np.float32)
    affine_w = np.asarray(affine_w, np.float32)
    affine_b = np.asarray(affine_b, np.float32)

    s = w @ affine_w.T + affine_b + 1.0  # (B, CIN)
    wsq = (weight.astype(np.float64) ** 2).sum(axis=(2, 3))  # (COUT, CIN)
    d = 1.0 / np.sqrt((s.astype(np.float64) ** 2) @ wsq.T + 1e-8)  # (B, COUT)
    d16 = (d / 16.0).astype(np.float32)

    xp = np.zeros((B, CIN, 34, 34), np.float32)
    xp[:, :, 1:33, 1:33] = x * s[:, :, None, None]
    xp_bf = xp.astype(ml_dtypes.bfloat16).reshape(B, NCI, P, 34 * 34)

    wf = weight[:, :, ::-1, ::-1]  # spatial flip
    wt = np.ascontiguousarray(
        wf.transpose(1, 2, 3, 0).reshape(NCI, P, 9 * COUT)
    ).astype(ml_dtypes.bfloat16)

    in_maps = []
    for core in range(NCORES):
        sl = slice(core * BPC, (core + 1) * BPC)
        dcore = d16[sl].reshape(BPC, NCO, P)
        dsc = np.ascontiguousarray(dcore.transpose(2, 0, 1).reshape(P, BPC * NCO))
        in_maps.append(
            {
                "xp": np.ascontiguousarray(xp_bf[sl]),
                "wt": wt,
                "dsc": dsc,
            }
        )
    return in_maps


LAST_RESULTS = None  # BassKernelResults of the most recent run (for test harness)


def kernel(x, w, weight, bias, affine_w, affine_b):
    global LAST_RESULTS
    in_maps = make_in_maps(x, w, weight, bias, affine_w, affine_b)
    nc = build_program()
    res = run_bass_kernel_spmd(nc, in_maps, list(range(NCORES)))
    LAST_RESULTS = res
    outs = [
        np.asarray(r["out"]).astype(np.float32).reshape(BPC, COUT, 64, 64)
        for r in res.results
    ]
    full = np.concatenate(outs, axis=0)
    full += np.asarray(bias, np.float32)  # (1, COUT, 1, 1) broadcast
    return np.ascontiguousarray(full, dtype=np.float32)
